# revision 18
# baseline (speedup 1.0000x reference)
"""AdaptiveGraphConv (Chebyshev K=3 graph conv) on 8 TRN2 NeuronCores.

Row-sharded over the 4096 nodes: core k owns nodes [512k, 512(k+1)).

Math (S = diag(s), s = d^-1/2 masked, A binary adj, L = I - S A S):
  out = h(W0+W1+W2) - Y(W1+4W2) + 2 Z W2,  Y = S A S h,  Z = (S A S)^2 h.
Evaluated as: p1n = h(W0+W1+W2) (entry, wcat block1)
  G = A (s*h) (MM1, rhs = host-staged fp8 xq), mix = (s*G) @ W2neg (epi1),
  p1n += mix;  U2 = s*(P1+2P2) + s*mix staged fp8 + AllGathered (u_a =
  s*(P1+2P2) entry-computed from wcat block0);  Z3 = A U2 (MM2);
  out = p1n - s*Z3.

v3 design notes (trace-driven; v1 236us -> v2 193us -> v3):
 - The Tile scheduler orders each in-order engine queue by SIMULATED
   readiness; priority only breaks ties. So ordering is controlled
   structurally: entry blocks for chunk fi+1 are issued INSIDE chunk fi's
   j-loop (one per j) so there is no entry backlog competing with the epi1
   trigger chain at chunk end, and the PE stream stays dense (HAM stays at
   K=8/8 instead of re-throttling to half clock each chunk boundary).
 - Both DMA rings share the ~340GB/s per-core HBM port: prefetching xq
   f1/f2 on sync in parallel (v2) starved the f0-critical stream and
   delayed MM1 f0 by ~25us. v3: scalar ring = abf/xq-f0 quarter-
   interleaved, then xq f1/f2, then the AllGather-output reads (uh).
   sync ring = consts + xcb, then per-chunk AG staging, then out.
 - AG staging moved from gpsimd sw-DGE (3-5.4us per 256KB, and the queue
   blocked the next trigger behind the in-flight collective) to the sync
   HW ring (~0.7us); gpsimd carries ONLY the 3 collective triggers.
 - wcat block1 refolded to W0+W1+W2 on host: pX (P0 buffer) deleted --
   entry loses 16 vector casts/chunk, epi2 loses 4 vector adds/chunk.
 - Lazy p1n += psM is one vector tensor_tensor from PSUM; epi2 is one STT
   + immediate per-mj out DMA on sync.
 - fp8 DoubleRow (k=256/pass) for both A-passes; gathered-node order is
   (core, p, mj) with host-permuted adjT/xq contraction rows so staging
   and uh reads are contiguous.
"""

from contextlib import ExitStack

import ml_dtypes
import numpy as np

import concourse.bacc as bacc
import concourse.mybir as mybir
import concourse.tile as tile
from concourse.bass_utils import run_bass_kernel_spmd
from concourse.masks import make_identity

P = 128
NCORES = 8
N = 4096
S = N // NCORES          # 512 nodes per core
B, C, T = 4, 32, 12
F = B * C * T            # 1536 flattened (t, bc) columns: f = 128*t + 32*b + c
NT = S * T               # 6144 free columns
KT = N // P              # 32 contraction tiles
MJ = S // P              # 4 node tiles per core
FB = 512                 # matmul moving-free block
NFB = F // FB            # 3
KPP = KT // MJ           # 8 ki-tiles per streamed MM2 quarter
TB = T // NFB            # 4 time steps per F chunk

f32 = mybir.dt.float32
bf16 = mybir.dt.bfloat16
fp8 = mybir.dt.float8e4
ALU = mybir.AluOpType
ACT_FN = mybir.ActivationFunctionType
DR = mybir.MatmulPerfMode.DoubleRow

_CACHE = {}


def _graph_kernel(ctx, tc, xs, xq, adjT, wc, sv, out):
    nc = tc.nc
    RG = [list(range(NCORES))]

    consts = ctx.enter_context(tc.tile_pool(name="consts", bufs=1))
    persist = ctx.enter_context(tc.tile_pool(name="persist", bufs=1))
    scratch = ctx.enter_context(tc.tile_pool(name="scratch", bufs=8))
    psum = ctx.enter_context(tc.tile_pool(name="psum", bufs=1, space="PSUM"))
    dram = ctx.enter_context(tc.tile_pool(name="dram", bufs=1, space="DRAM"))

    # ---------------- DMA rings (see module docstring)
    wcb = consts.tile([P, 3 * P], bf16)     # [wcat | w2neg] packed
    nc.sync.dma_start(wcb[:], wc[:])
    wcat = wcb[:, 0:2 * P]
    w2neg = wcb[:, 2 * P:3 * P]
    svals = consts.tile([P, 2 * MJ], f32)   # [ s | -s ] for own shard
    nc.sync.dma_start(svals[:], sv[:])
    xcb = persist.tile([P, T, MJ, P], bf16)
    xsv = xs.rearrange("p (t m n) -> p t m n", t=T, m=MJ)
    for fi in range(NFB):
        tsl = slice(TB * fi, TB * (fi + 1))
        nc.sync.dma_start(xcb[:, tsl], xsv[:, tsl])

    abf = persist.tile([P, KT, S], fp8)
    abv = adjT.rearrange("p (k m) -> p k m", k=KT)
    xqb = persist.tile([P, NFB, KT, FB], fp8)
    xqv = xq.rearrange("p (c k f) -> p c k f", c=NFB, k=KT)
    for q in range(MJ):
        ksl = slice(KPP * q, KPP * (q + 1))
        nc.scalar.dma_start(abf[:, ksl, :], abv[:, ksl, :])
        nc.scalar.dma_start(xqb[:, 0, ksl, :], xqv[:, 0, ksl, :])
    for fi in range(1, NFB):
        nc.scalar.dma_start(xqb[:, fi], xqv[:, fi])
    ident = consts.tile([P, P], bf16)
    make_identity(nc, ident[:])

    # ---------------- node-major state: [p, mj, f], n_local = 128*mj + p
    p1n = persist.tile([P, MJ, F], f32)       # h(W0+W1+W2) -> +mix -> out
    u_a = persist.tile([P, MJ, T, P], bf16)   # s*(P1+2*P2), entry-computed
    ustage = persist.tile([P, MJ, F], fp8)    # AG staging U2 = s*M
    p1n_v = p1n.rearrange("p m (t o) -> p m t o", t=T)
    u_a_f = u_a.rearrange("p m t o -> p m (t o)")

    # ---------------- entry: one (mj, t) block = one psE matmul + p1n copy
    # (vector) + u_a scale (scalar). Chunk f0 runs up-front (warms the HAM
    # clock gate while the abf/xq DMAs stream); chunks f1/f2 are issued one
    # block per j-iteration inside the previous chunk's pass loop.
    # PSUM slots are bank-padded (2KB): two entry psE blocks share one
    # bank tile so tag "pe" (bufs=2) gives 4 blocks of slack in 2 banks --
    # entry blocks interleaved into the pass stream never stall it.
    _pair = {}

    def entry_block(fi, b, mj, t):
        if b % 2 == 0:
            _pair[0] = psum.tile([P, 2, 2 * P], f32, tag="pe", bufs=3,
                                 name=f"psE_{fi}_{b // 2}")
        psE = _pair[0][:, b % 2, :]
        # wcat = [W1+2*W2 | W0+W1+W2] prefolded on host
        nc.tensor.matmul(psE, xcb[:, t, mj, :], wcat,
                         start=True, stop=True)
        nc.vector.tensor_copy(p1n_v[:, mj, t, :], psE[:, P:2 * P])
        nc.scalar.activation(u_a[:, mj, t, :], psE[:, 0:P],
                             ACT_FN.Identity, scale=svals[:, mj:mj + 1])

    def entry_blocks_of(fi):
        return [(mj, t) for mj in range(MJ)
                for t in range(TB * fi, TB * (fi + 1))]

    with tc.high_priority():
        for b, (mj, t) in enumerate(entry_blocks_of(0)):
            entry_block(0, b, mj, t)

    ag_out = [None] * NFB

    # Keep-warm filler: the AllGather-gated gaps between MM2 chunks are
    # 10-13us of PE idle, which re-throttles the HAM clock gate to half
    # rate (cold DR passes are 480ns vs 259ns warm). A paced chain of
    # [PE transpose -> vector copy] pairs (rotation on the idle "pe" PSUM
    # slots, reader-paced at ~0.35us/pair) keeps PE activity above the
    # MID-window threshold without delaying the next chunk's passes.
    _wf = {"n": 0}

    def warm_fill(n):
        for _ in range(n):
            k = _wf["n"] = _wf["n"] + 1
            psD = psum.tile([P, 2, 2 * P], bf16, tag="pe", bufs=3,
                            name=f"psD_{k}")
            nc.tensor.transpose(psD[:, 0, 0:P], ident[:], ident[:])
            vd = scratch.tile([P, P], bf16, tag="vd", bufs=2,
                              name=f"vd_{k}")
            nc.vector.tensor_copy(vd[:], psD[:, 0, 0:P])

    def mm_pass(rhs_of, tag, epilogue, inter=None, post=None):
        # rhs_of(fi) -> [P, KT, FB] fp8 SBUF view; 4 psum banks (one per
        # mj) accumulate over 16 DoubleRow passes (k=256 each). j-outer so
        # compute paces with the ki-quarter DMAs of the rhs instead of
        # waiting for the full chunk; inter(fi, j) issues filler work
        # (next chunk's entry blocks) inside the pass stream.
        rhss = {0: rhs_of(0)}
        for fi in range(NFB):
            if fi + 1 < NFB:
                rhss[fi + 1] = rhs_of(fi + 1)
            rhs = rhss[fi]
            pms = [psum.tile([P, FB], f32, tag="pm", bufs=5,
                             name=f"pm_{tag}_{fi}_{mj}") for mj in range(MJ)]
            for j in range(KT // 2):
                for mj in range(MJ):
                    nc.tensor.matmul(
                        pms[mj][:],
                        abf[:, 2 * j:2 * j + 2, P * mj:P * (mj + 1)],
                        rhs[:, 2 * j:2 * j + 2, :], perf_mode=DR,
                        start=(j == 0), stop=(j == KT // 2 - 1))
                if inter is not None:
                    inter(fi, j)
            epilogue(fi, pms)
            if post is not None:
                post(fi)

    # ---------------- MM1: G = A (S h); mix = (s*G) @ W2neg; stage
    # U2 = u_a + s*mix on the sync ring and fire this chunk's AllGather
    # from gpsimd. The whole trigger chain runs at high priority; the
    # p1n += psM update that epi2 needs is LAZY -- issued at normal
    # priority after the trigger.
    def epi1(fi, pms):
        fsl = slice(FB * fi, FB * (fi + 1))
        tsl = slice(TB * fi, TB * (fi + 1))
        psms, vgs, vTs = [], [], []
        with tc.high_priority():
            # phase-batched: all vg first (frees the pm banks earliest so
            # the next chunk's passes start immediately), then psT / vT /
            # psM / STT -- each mj's scalar latency hides behind the other
            # mjs' PE work.
            for mj in range(MJ):
                vg = scratch.tile([P, TB, P], bf16, tag="vg", bufs=4,
                                  name=f"vg_{fi}_{mj}")
                nc.scalar.activation(vg.rearrange("p t o -> p (t o)"),
                                     pms[mj][:], ACT_FN.Identity,
                                     scale=svals[:, mj:mj + 1])
                vgs.append(vg)
            # psT pairs share the "pmx" tag with psM: psM mj0's slot is
            # freed by vT mj0/mj1 just before it, and the next chunk's
            # psT pair by psM mj2's readers -- no coupling into the pass
            # stream and the PSUM budget stays at 8 banks.
            psTp = [psum.tile([P, 2, TB, P], bf16, tag="pm", bufs=5,
                              name=f"psT_{fi}_{pr}") for pr in range(2)]
            for mj in range(MJ):
                for j in range(TB):
                    nc.tensor.transpose(psTp[mj // 2][:, mj % 2, j, :],
                                        vgs[mj][:, j, :], ident[:])
            for mj in range(MJ):
                vT = scratch.tile([P, TB, P], bf16, tag="vT", bufs=4,
                                  name=f"vT_{fi}_{mj}")
                nc.scalar.copy(vT[:], psTp[mj // 2][:, mj % 2])
                vTs.append(vT)
            psms = [psum.tile([P, TB, P], f32, tag="pm", bufs=5,
                              name=f"psM_{fi}_{mj}") for mj in range(MJ)]
            for mj in range(MJ):
                for j in range(TB):
                    nc.tensor.matmul(psms[mj][:, j, :], vTs[mj][:, j, :],
                                     w2neg, start=True, stop=True)
            for mj in range(MJ):
                nc.vector.scalar_tensor_tensor(
                    ustage[:, mj, fsl],
                    psms[mj].rearrange("p t o -> p (t o)"),
                    svals[:, mj:mj + 1],
                    u_a_f[:, mj, fsl], op0=ALU.mult, op1=ALU.add)
            # gathered-node ordering is (core, p, mj) -- the host permutes
            # the contraction-row order of adjT/xq to match -- so this
            # staging DMA is contiguous 2KB lines per partition.
            agi = dram.tile([P * MJ, FB], fp8, name=f"ag2i{fi}")
            ago = dram.tile([N, FB], fp8, addr_space="Shared",
                            name=f"ag2o{fi}")
            nc.sync.dma_start(agi.rearrange("(p m) f -> p m f", p=P),
                              ustage[:, :, fsl])
            nc.gpsimd.collective_compute(
                "AllGather", ALU.bypass, replica_groups=RG,
                ins=[agi.opt()], outs=[ago.opt()],
            )
            # p1n += mix, in-chain (vector, after the STTs) so the 2 psM
            # slots recycle promptly; nothing downstream waits on these.
            for mj in range(MJ):
                nc.vector.tensor_tensor(
                    p1n_v[:, mj, tsl, :], psms[mj][:],
                    p1n_v[:, mj, tsl, :], op=ALU.add)
        ag_out[fi] = ago

    def inter1(fi, j):
        if fi + 1 < NFB:
            blocks = entry_blocks_of(fi + 1)
            entry_block(fi + 1, j, *blocks[j])

    mm_pass(lambda fi: xqb[:, fi], "g", epi1, inter=inter1,
            post=lambda fi: warm_fill(30) if fi == NFB - 1 else None)

    # ---------------- MM2: Z3 = A U2; out = p1n - s*Z3; exit fused
    def uh_of(fi):
        # tile_wait_until stamps a LOGICAL (sim-only) ready time: the Tile
        # scheduler's CC cost model is optimistic, and without this it
        # statically orders AllGather-gated MM2 work ahead of the later
        # epi1 trigger chains on the PE/scalar streams, serializing the
        # last AllGather behind all of MM2 f0 on real hardware.
        uh = scratch.tile([P, KT, FB], fp8, tag="uh", bufs=2, name=f"uh_{fi}")
        agv = ag_out[fi].rearrange("(ki p) f -> p ki f", p=P)
        with tc.tile_wait_until(0.2 + 0.01 * fi):
            for q in range(MJ):
                ksl = slice(KPP * q, KPP * (q + 1))
                nc.scalar.dma_start(uh[:, ksl, :], agv[:, ksl, :])
        return uh

    # out stays node-major [p, mj, f] f32 -- the host unshard transposes
    # back to [B, C, N, T] and adds the bias for free.
    outv = out.rearrange("p (m f) -> p m f", m=MJ)

    def epi2(fi, pms):
        fsl = slice(FB * fi, FB * (fi + 1))
        for mj in range(MJ):
            nc.vector.scalar_tensor_tensor(
                p1n[:, mj, fsl], pms[mj][:], svals[:, MJ + mj:MJ + mj + 1],
                p1n[:, mj, fsl], op0=ALU.mult, op1=ALU.add)
            nc.sync.dma_start(outv[:, mj, fsl], p1n[:, mj, fsl])

    mm_pass(uh_of, "z3", epi2,
            post=lambda fi: warm_fill(30) if fi < NFB - 1 else None)


def build_nc():
    nc = bacc.Bacc(target_bir_lowering=False)
    xs = nc.declare_dram_parameter("xs", [P, NT], bf16, isOutput=False)
    xq = nc.declare_dram_parameter("xq", [P, NFB * KT * FB], fp8,
                                   isOutput=False)
    adjT = nc.declare_dram_parameter("adjT", [P, KT * S], fp8, isOutput=False)
    wc = nc.declare_dram_parameter("wcb", [P, 3 * P], bf16, isOutput=False)
    sv = nc.declare_dram_parameter("svals", [P, 2 * MJ], f32, isOutput=False)
    out = nc.declare_dram_parameter("out", [P, MJ * F], f32, isOutput=True)
    with tile.TileContext(nc) as tc, ExitStack() as ctx:
        _graph_kernel(ctx, tc, xs, xq, adjT, wc, sv, out)
    nc.compile()
    return nc


def make_in_maps(x, adj, weight, bias):
    wcb = np.zeros((P, 3 * P), np.float32)
    mats = [weight[1] + 2.0 * weight[2], weight[0] + weight[1] + weight[2],
            -2.0 * weight[2]]
    for j, m in enumerate(mats):
        for b in range(B):
            wcb[32 * b:32 * (b + 1), P * j + 32 * b:P * j + 32 * (b + 1)] = m
    wcb = wcb.astype(ml_dtypes.bfloat16)

    d = adj.sum(axis=1)
    s = np.where(d > 0, 1.0 / np.sqrt(np.maximum(d, 1.0)), 0.0).astype(
        np.float32)
    # The AllGather output rows land in (core, p, mj) order (contiguous
    # device staging); permute the contraction-row order of xq and adjT to
    # match: contraction position 512c + 4p + mj holds node 512c + 128mj + p.
    lperm = (np.arange(MJ)[None, :] * P + np.arange(P)[:, None]).reshape(-1)
    rperm = (np.arange(NCORES)[:, None] * S + lperm[None, :]).reshape(-1)
    # xq[p, fc, ki, fb]: fp8 s*x, contraction row = 128*ki + p,
    # f = 512*fc + fb enumerates (t, b, c) = 128*t + 32*b + c. Replicated.
    xq = (x * s[None, None, :, None]).transpose(2, 3, 0, 1)  # [N, T, B, C]
    xq = xq[rperm].reshape(KT, P, F).transpose(1, 0, 2)      # [p, ki, f]
    xq = np.ascontiguousarray(
        xq.reshape(P, KT, NFB, FB).transpose(0, 2, 1, 3)).reshape(
            P, NFB * KT * FB).astype(ml_dtypes.float8_e4m3)
    adjp = adj[rperm]

    in_maps = []
    for k in range(NCORES):
        sl = slice(S * k, S * (k + 1))
        xsb = np.ascontiguousarray(
            x[:, :, sl, :].reshape(P, MJ, P, T).transpose(0, 3, 1, 2)
        ).reshape(P, NT).astype(ml_dtypes.bfloat16)
        adjb = np.ascontiguousarray(
            adjp[:, sl].reshape(KT, P, S).transpose(1, 0, 2)).reshape(
                P, KT * S).astype(ml_dtypes.float8_e4m3)
        sk = s[sl].reshape(MJ, P).T  # [p, mj]
        svals = np.concatenate([sk, -sk], axis=1).astype(np.float32)
        in_maps.append({
            "xs": xsb,
            "xq": xq,
            "adjT": adjb,
            "wcb": wcb,
            "svals": svals,
        })
    return in_maps


def kernel(x, adj, weight, bias, _trace=False, _tmpdir=None):
    if "nc" not in _CACHE:
        _CACHE["nc"] = build_nc()
    nc = _CACHE["nc"]
    in_maps = make_in_maps(
        np.asarray(x, np.float32), np.asarray(adj, np.float32),
        np.asarray(weight, np.float32), np.asarray(bias, np.float32))
    res = run_bass_kernel_spmd(nc, in_maps, core_ids=list(range(NCORES)),
                               trace=_trace, tmpdir=_tmpdir)
    _CACHE["last_result"] = res
    # node-major [p, mj, t, b, o] -> [B, C, S, T] per core; bias on host
    parts = [r["out"].reshape(P, MJ, T, B, 32).transpose(3, 4, 1, 0, 2)
             .reshape(B, C, S, T) for r in res.results]
    full = np.concatenate(parts, axis=2)
    full = full + np.asarray(bias, np.float32)[None, :, None, None]
    return np.ascontiguousarray(full)


# revision 23
# speedup vs baseline: 1.0047x; 1.0047x over previous
"""AdaptiveGraphConv (Chebyshev K=3 graph conv) on 8 TRN2 NeuronCores.

Row-sharded over the 4096 nodes: core k owns nodes [512k, 512(k+1)).

Math (S = diag(s), s = d^-1/2 masked, A binary adj, L = I - S A S):
  out = h(W0+W1+W2) - Y(W1+4W2) + 2 Z W2,  Y = S A S h,  Z = (S A S)^2 h.
Evaluated as: p1n = h(W0+W1+W2) (entry, wcat block1)
  G = A (s*h) (MM1, rhs = host-staged fp8 xq), mix = (s*G) @ W2neg (epi1),
  p1n += mix;  U2 = s*(P1+2P2) + s*mix staged fp8 + AllGathered (u_a =
  s*(P1+2P2) entry-computed from wcat block0);  Z3 = A U2 (MM2);
  out = p1n - s*Z3.

v3 design notes (trace-driven; v1 236us -> v2 193us -> v3):
 - The Tile scheduler orders each in-order engine queue by SIMULATED
   readiness; priority only breaks ties. So ordering is controlled
   structurally: entry blocks for chunk fi+1 are issued INSIDE chunk fi's
   j-loop (one per j) so there is no entry backlog competing with the epi1
   trigger chain at chunk end, and the PE stream stays dense (HAM stays at
   K=8/8 instead of re-throttling to half clock each chunk boundary).
 - Both DMA rings share the ~340GB/s per-core HBM port: prefetching xq
   f1/f2 on sync in parallel (v2) starved the f0-critical stream and
   delayed MM1 f0 by ~25us. v3: scalar ring = abf/xq-f0 quarter-
   interleaved, then xq f1/f2, then the AllGather-output reads (uh).
   sync ring = consts + xcb, then per-chunk AG staging, then out.
 - AG staging moved from gpsimd sw-DGE (3-5.4us per 256KB, and the queue
   blocked the next trigger behind the in-flight collective) to the sync
   HW ring (~0.7us); gpsimd carries ONLY the 3 collective triggers.
 - wcat block1 refolded to W0+W1+W2 on host: pX (P0 buffer) deleted --
   entry loses 16 vector casts/chunk, epi2 loses 4 vector adds/chunk.
 - Lazy p1n += psM is one vector tensor_tensor from PSUM; epi2 is one STT
   + immediate per-mj out DMA on sync.
 - fp8 DoubleRow (k=256/pass) for both A-passes; gathered-node order is
   (core, p, mj) with host-permuted adjT/xq contraction rows so staging
   and uh reads are contiguous.
"""

from contextlib import ExitStack

import ml_dtypes
import numpy as np

import concourse.bacc as bacc
import concourse.mybir as mybir
import concourse.tile as tile
from concourse.bass_utils import run_bass_kernel_spmd
from concourse.masks import make_identity

P = 128
NCORES = 8
N = 4096
S = N // NCORES          # 512 nodes per core
B, C, T = 4, 32, 12
F = B * C * T            # 1536 flattened (t, bc) columns: f = 128*t + 32*b + c
NT = S * T               # 6144 free columns
KT = N // P              # 32 contraction tiles
MJ = S // P              # 4 node tiles per core
FB = 512                 # matmul moving-free block
NFB = F // FB            # 3
KPP = KT // MJ           # 8 ki-tiles per streamed MM2 quarter
TB = T // NFB            # 4 time steps per F chunk

f32 = mybir.dt.float32
bf16 = mybir.dt.bfloat16
fp8 = mybir.dt.float8e4
ALU = mybir.AluOpType
ACT_FN = mybir.ActivationFunctionType
DR = mybir.MatmulPerfMode.DoubleRow

_CACHE = {}


def _graph_kernel(ctx, tc, xs, xq, adjT, wc, sv, out):
    nc = tc.nc
    RG = [list(range(NCORES))]

    consts = ctx.enter_context(tc.tile_pool(name="consts", bufs=1))
    persist = ctx.enter_context(tc.tile_pool(name="persist", bufs=1))
    scratch = ctx.enter_context(tc.tile_pool(name="scratch", bufs=8))
    psum = ctx.enter_context(tc.tile_pool(name="psum", bufs=1, space="PSUM"))
    dram = ctx.enter_context(tc.tile_pool(name="dram", bufs=1, space="DRAM"))

    # ---------------- DMA rings (see module docstring)
    wcb = consts.tile([P, 3 * P], bf16)     # [wcat | w2neg] packed
    nc.sync.dma_start(wcb[:], wc[:])
    wcat = wcb[:, 0:2 * P]
    w2neg = wcb[:, 2 * P:3 * P]
    svals = consts.tile([P, 2 * MJ], f32)   # [ s | -s ] for own shard
    nc.sync.dma_start(svals[:], sv[:])
    xcb = persist.tile([P, T, MJ, P], bf16)
    xsv = xs.rearrange("p (t m n) -> p t m n", t=T, m=MJ)
    for fi in range(NFB):
        tsl = slice(TB * fi, TB * (fi + 1))
        nc.sync.dma_start(xcb[:, tsl], xsv[:, tsl])

    abf = persist.tile([P, KT, S], fp8)
    abv = adjT.rearrange("p (k m) -> p k m", k=KT)
    xqb = persist.tile([P, NFB, KT, FB], fp8)
    xqv = xq.rearrange("p (c k f) -> p c k f", c=NFB, k=KT)
    for q in range(MJ):
        ksl = slice(KPP * q, KPP * (q + 1))
        nc.scalar.dma_start(abf[:, ksl, :], abv[:, ksl, :])
        nc.scalar.dma_start(xqb[:, 0, ksl, :], xqv[:, 0, ksl, :])
    for fi in range(1, NFB):
        nc.scalar.dma_start(xqb[:, fi], xqv[:, fi])
    ident = consts.tile([P, P], bf16)
    make_identity(nc, ident[:])

    # ---------------- node-major state: [p, mj, f], n_local = 128*mj + p
    p1n = persist.tile([P, MJ, F], f32)       # h(W0+W1+W2) -> +mix -> out
    u_a = persist.tile([P, MJ, T, P], bf16)   # s*(P1+2*P2), entry-computed
    ustage = persist.tile([P, MJ, F], fp8)    # AG staging U2 = s*M
    p1n_v = p1n.rearrange("p m (t o) -> p m t o", t=T)
    u_a_f = u_a.rearrange("p m t o -> p m (t o)")

    # ---------------- entry: one (mj, t) block = one psE matmul + p1n copy
    # (vector) + u_a scale (scalar). Chunk f0 runs up-front (warms the HAM
    # clock gate while the abf/xq DMAs stream); chunks f1/f2 are issued one
    # block per j-iteration inside the previous chunk's pass loop.
    # PSUM slots are bank-padded (2KB): two entry psE blocks share one
    # bank tile so tag "pe" (bufs=2) gives 4 blocks of slack in 2 banks --
    # entry blocks interleaved into the pass stream never stall it.
    _pair = {}

    def entry_block(fi, b, mj, t):
        if b % 2 == 0:
            _pair[0] = psum.tile([P, 2, 2 * P], f32, tag="pe", bufs=3,
                                 name=f"psE_{fi}_{b // 2}")
        psE = _pair[0][:, b % 2, :]
        # wcat = [W1+2*W2 | W0+W1+W2] prefolded on host
        nc.tensor.matmul(psE, xcb[:, t, mj, :], wcat,
                         start=True, stop=True)
        nc.vector.tensor_copy(p1n_v[:, mj, t, :], psE[:, P:2 * P])
        nc.scalar.activation(u_a[:, mj, t, :], psE[:, 0:P],
                             ACT_FN.Identity, scale=svals[:, mj:mj + 1])

    def entry_blocks_of(fi):
        return [(mj, t) for mj in range(MJ)
                for t in range(TB * fi, TB * (fi + 1))]

    with tc.high_priority():
        for b, (mj, t) in enumerate(entry_blocks_of(0)):
            entry_block(0, b, mj, t)

    ag_out = [None] * NFB

    # Keep-warm ticks: the AllGather-gated gaps between MM2 chunks are
    # 10-19us of PE idle, which re-throttles the HAM clock gate to half
    # rate (cold DR passes are 480ns vs 259ns warm). One tiny PE matmul
    # every ~2.5us -- dep-paced by a serial chain of dead vector copies --
    # keeps PE activity inside every HAM MID window. If the real work
    # lands mid-chain, leftover ticks have their deps met and cost ~0.1us
    # each on the in-order PE queue.
    _wf = {"n": 0}

    def warm_ticks(n_ticks, chain_len=8):
        vda = scratch.tile([P, P], bf16, tag="vd", bufs=1, name="vda")
        vdb = scratch.tile([P, P], bf16, tag="vd2", bufs=1, name="vdb")
        vdc = scratch.tile([P, P], f32, tag="vd3", bufs=1, name="vdc")
        nc.vector.tensor_copy(vda[:], ident[:])
        psD = None
        for _ in range(n_ticks):
            k = _wf["n"] = _wf["n"] + 1
            if psD is not None:
                nc.vector.tensor_copy(vdc[:], psD[:, 0, 0:P])
            for c in range(chain_len):
                src, dst = (vda, vdb) if c % 2 == 0 else (vdb, vda)
                nc.vector.tensor_copy(dst[:], src[:])
            psD = psum.tile([P, 2, 2 * P], f32, tag="pe", bufs=3,
                            name=f"psD_{k}")
            nc.tensor.matmul(psD[:, 0, 0:P],
                             vda[:] if chain_len % 2 == 0 else vdb[:],
                             ident[:], start=True, stop=True)
        nc.vector.tensor_copy(vdc[:], psD[:, 0, 0:P])

    def mm_pass(rhs_of, tag, epilogue, inter=None, post=None):
        # rhs_of(fi) -> [P, KT, FB] fp8 SBUF view; 4 psum banks (one per
        # mj) accumulate over 16 DoubleRow passes (k=256 each). j-outer so
        # compute paces with the ki-quarter DMAs of the rhs instead of
        # waiting for the full chunk; inter(fi, j) issues filler work
        # (next chunk's entry blocks) inside the pass stream.
        rhss = {0: rhs_of(0)}
        for fi in range(NFB):
            if fi + 1 < NFB:
                rhss[fi + 1] = rhs_of(fi + 1)
            rhs = rhss[fi]
            pms = [psum.tile([P, FB], f32, tag="pm", bufs=5,
                             name=f"pm_{tag}_{fi}_{mj}") for mj in range(MJ)]
            for j in range(KT // 2):
                for mj in range(MJ):
                    nc.tensor.matmul(
                        pms[mj][:],
                        abf[:, 2 * j:2 * j + 2, P * mj:P * (mj + 1)],
                        rhs[:, 2 * j:2 * j + 2, :], perf_mode=DR,
                        start=(j == 0), stop=(j == KT // 2 - 1))
                if inter is not None:
                    inter(fi, j)
            epilogue(fi, pms)
            if post is not None:
                post(fi)

    # ---------------- MM1: G = A (S h); mix = (s*G) @ W2neg; stage
    # U2 = u_a + s*mix on the sync ring and fire this chunk's AllGather
    # from gpsimd. The whole trigger chain runs at high priority; the
    # p1n += psM update that epi2 needs is LAZY -- issued at normal
    # priority after the trigger.
    def epi1(fi, pms):
        fsl = slice(FB * fi, FB * (fi + 1))
        tsl = slice(TB * fi, TB * (fi + 1))
        psms, vgs, vTs = [], [], []
        with tc.high_priority():
            # phase-batched: all vg first (frees the pm banks earliest so
            # the next chunk's passes start immediately), then psT / vT /
            # psM / STT -- each mj's scalar latency hides behind the other
            # mjs' PE work.
            for mj in range(MJ):
                vg = scratch.tile([P, TB, P], bf16, tag="vg", bufs=4,
                                  name=f"vg_{fi}_{mj}")
                nc.scalar.activation(vg.rearrange("p t o -> p (t o)"),
                                     pms[mj][:], ACT_FN.Identity,
                                     scale=svals[:, mj:mj + 1])
                vgs.append(vg)
            # psT pairs share the "pmx" tag with psM: psM mj0's slot is
            # freed by vT mj0/mj1 just before it, and the next chunk's
            # psT pair by psM mj2's readers -- no coupling into the pass
            # stream and the PSUM budget stays at 8 banks.
            psTp = [psum.tile([P, 2, TB, P], bf16, tag="pm", bufs=5,
                              name=f"psT_{fi}_{pr}") for pr in range(2)]
            for mj in range(MJ):
                for j in range(TB):
                    nc.tensor.transpose(psTp[mj // 2][:, mj % 2, j, :],
                                        vgs[mj][:, j, :], ident[:])
            for mj in range(MJ):
                vT = scratch.tile([P, TB, P], bf16, tag="vT", bufs=4,
                                  name=f"vT_{fi}_{mj}")
                nc.scalar.copy(vT[:], psTp[mj // 2][:, mj % 2])
                vTs.append(vT)
            psms = [psum.tile([P, TB, P], f32, tag="pm", bufs=5,
                              name=f"psM_{fi}_{mj}") for mj in range(MJ)]
            for mj in range(MJ):
                for j in range(TB):
                    nc.tensor.matmul(psms[mj][:, j, :], vTs[mj][:, j, :],
                                     w2neg, start=True, stop=True)
            for mj in range(MJ):
                nc.vector.scalar_tensor_tensor(
                    ustage[:, mj, fsl],
                    psms[mj].rearrange("p t o -> p (t o)"),
                    svals[:, mj:mj + 1],
                    u_a_f[:, mj, fsl], op0=ALU.mult, op1=ALU.add)
            # gathered-node ordering is (core, p, mj) -- the host permutes
            # the contraction-row order of adjT/xq to match -- so this
            # staging DMA is contiguous 2KB lines per partition.
            agi = dram.tile([P * MJ, FB], fp8, name=f"ag2i{fi}")
            ago = dram.tile([N, FB], fp8, addr_space="Shared",
                            name=f"ag2o{fi}")
            nc.sync.dma_start(agi.rearrange("(p m) f -> p m f", p=P),
                              ustage[:, :, fsl])
            nc.gpsimd.collective_compute(
                "AllGather", ALU.bypass, replica_groups=RG,
                ins=[agi.opt()], outs=[ago.opt()],
            )
            # p1n += mix, in-chain (vector, after the STTs) so the 2 psM
            # slots recycle promptly; nothing downstream waits on these.
            for mj in range(MJ):
                nc.vector.tensor_tensor(
                    p1n_v[:, mj, tsl, :], psms[mj][:],
                    p1n_v[:, mj, tsl, :], op=ALU.add)
        ag_out[fi] = ago

    def inter1(fi, j):
        if fi + 1 < NFB:
            blocks = entry_blocks_of(fi + 1)
            entry_block(fi + 1, j, *blocks[j])

    mm_pass(lambda fi: xqb[:, fi], "g", epi1, inter=inter1,
            post=lambda fi: warm_ticks(5) if fi == NFB - 1 else None)

    # ---------------- MM2: Z3 = A U2; out = p1n - s*Z3; exit fused
    def uh_of(fi):
        # tile_wait_until stamps a LOGICAL (sim-only) ready time: the Tile
        # scheduler's CC cost model is optimistic, and without this it
        # statically orders AllGather-gated MM2 work ahead of the later
        # epi1 trigger chains on the PE/scalar streams, serializing the
        # last AllGather behind all of MM2 f0 on real hardware.
        uh = scratch.tile([P, KT, FB], fp8, tag="uh", bufs=2, name=f"uh_{fi}")
        agv = ag_out[fi].rearrange("(ki p) f -> p ki f", p=P)
        with tc.tile_wait_until(0.2 + 0.01 * fi):
            for q in range(MJ):
                ksl = slice(KPP * q, KPP * (q + 1))
                nc.scalar.dma_start(uh[:, ksl, :], agv[:, ksl, :])
        return uh

    # out stays node-major [p, mj, f] f32 -- the host unshard transposes
    # back to [B, C, N, T] and adds the bias for free.
    outv = out.rearrange("p (m f) -> p m f", m=MJ)

    def epi2(fi, pms):
        fsl = slice(FB * fi, FB * (fi + 1))
        for mj in range(MJ):
            nc.vector.scalar_tensor_tensor(
                p1n[:, mj, fsl], pms[mj][:], svals[:, MJ + mj:MJ + mj + 1],
                p1n[:, mj, fsl], op0=ALU.mult, op1=ALU.add)
            nc.sync.dma_start(outv[:, mj, fsl], p1n[:, mj, fsl])

    mm_pass(uh_of, "z3", epi2,
            post=lambda fi: warm_ticks(8) if fi < NFB - 1 else None)


def build_nc():
    nc = bacc.Bacc(target_bir_lowering=False)
    xs = nc.declare_dram_parameter("xs", [P, NT], bf16, isOutput=False)
    xq = nc.declare_dram_parameter("xq", [P, NFB * KT * FB], fp8,
                                   isOutput=False)
    adjT = nc.declare_dram_parameter("adjT", [P, KT * S], fp8, isOutput=False)
    wc = nc.declare_dram_parameter("wcb", [P, 3 * P], bf16, isOutput=False)
    sv = nc.declare_dram_parameter("svals", [P, 2 * MJ], f32, isOutput=False)
    out = nc.declare_dram_parameter("out", [P, MJ * F], f32, isOutput=True)
    with tile.TileContext(nc) as tc, ExitStack() as ctx:
        _graph_kernel(ctx, tc, xs, xq, adjT, wc, sv, out)
    nc.compile()
    return nc


def make_in_maps(x, adj, weight, bias):
    wcb = np.zeros((P, 3 * P), np.float32)
    mats = [weight[1] + 2.0 * weight[2], weight[0] + weight[1] + weight[2],
            -2.0 * weight[2]]
    for j, m in enumerate(mats):
        for b in range(B):
            wcb[32 * b:32 * (b + 1), P * j + 32 * b:P * j + 32 * (b + 1)] = m
    wcb = wcb.astype(ml_dtypes.bfloat16)

    d = adj.sum(axis=1)
    s = np.where(d > 0, 1.0 / np.sqrt(np.maximum(d, 1.0)), 0.0).astype(
        np.float32)
    # The AllGather output rows land in (core, p, mj) order (contiguous
    # device staging); permute the contraction-row order of xq and adjT to
    # match: contraction position 512c + 4p + mj holds node 512c + 128mj + p.
    lperm = (np.arange(MJ)[None, :] * P + np.arange(P)[:, None]).reshape(-1)
    rperm = (np.arange(NCORES)[:, None] * S + lperm[None, :]).reshape(-1)
    # xq[p, fc, ki, fb]: fp8 s*x, contraction row = 128*ki + p,
    # f = 512*fc + fb enumerates (t, b, c) = 128*t + 32*b + c. Replicated.
    xq = (x * s[None, None, :, None]).transpose(2, 3, 0, 1)  # [N, T, B, C]
    xq = xq[rperm].reshape(KT, P, F).transpose(1, 0, 2)      # [p, ki, f]
    xq = np.ascontiguousarray(
        xq.reshape(P, KT, NFB, FB).transpose(0, 2, 1, 3)).reshape(
            P, NFB * KT * FB).astype(ml_dtypes.float8_e4m3)
    adjp = adj[rperm]

    in_maps = []
    for k in range(NCORES):
        sl = slice(S * k, S * (k + 1))
        xsb = np.ascontiguousarray(
            x[:, :, sl, :].reshape(P, MJ, P, T).transpose(0, 3, 1, 2)
        ).reshape(P, NT).astype(ml_dtypes.bfloat16)
        adjb = np.ascontiguousarray(
            adjp[:, sl].reshape(KT, P, S).transpose(1, 0, 2)).reshape(
                P, KT * S).astype(ml_dtypes.float8_e4m3)
        sk = s[sl].reshape(MJ, P).T  # [p, mj]
        svals = np.concatenate([sk, -sk], axis=1).astype(np.float32)
        in_maps.append({
            "xs": xsb,
            "xq": xq,
            "adjT": adjb,
            "wcb": wcb,
            "svals": svals,
        })
    return in_maps


def kernel(x, adj, weight, bias, _trace=False, _tmpdir=None):
    if "nc" not in _CACHE:
        _CACHE["nc"] = build_nc()
    nc = _CACHE["nc"]
    in_maps = make_in_maps(
        np.asarray(x, np.float32), np.asarray(adj, np.float32),
        np.asarray(weight, np.float32), np.asarray(bias, np.float32))
    res = run_bass_kernel_spmd(nc, in_maps, core_ids=list(range(NCORES)),
                               trace=_trace, tmpdir=_tmpdir)
    _CACHE["last_result"] = res
    # node-major [p, mj, t, b, o] -> [B, C, S, T] per core; bias on host
    parts = [r["out"].reshape(P, MJ, T, B, 32).transpose(3, 4, 1, 0, 2)
             .reshape(B, C, S, T) for r in res.results]
    full = np.concatenate(parts, axis=2)
    full = full + np.asarray(bias, np.float32)[None, :, None, None]
    return np.ascontiguousarray(full)
